# revision 7
# baseline (speedup 1.0000x reference)
"""CFG dual self-attention kernel for 8 Trainium2 NeuronCores — v2.

Redesign vs v1 (all-fp32r baseline):
  - bf16 operands everywhere (matmul cost identical per the TRN2 cost model;
    DMA bytes and SBUF footprint halve; DVE gets 2x/4x perf modes).  fp32
    only in PSUM accumulators, softmax statistics and the final output.
  - Phase 1a keeps Wq+Wk resident in SBUF and streams hT once for both
    (baseline re-read hT per projection); phase 1b streams hT once more
    for V with Wv resident (Wv streams in during the 1a tail).
  - ssq allreduce and the rmsnorm 1/sqrt chain run inside the V pass on
    single-partition rows; the factors spill to DRAM and are re-broadcast
    at attention start, so no SBUF pool needs to span the phase boundary.
  - Attention: scores for TWO key chunks land in one 2-bank PSUM tile and
    get a single exp (halves Act overhead); softmax column sums come from a
    DVE bf16 add-tree over the exp tiles plus ONE ones-matmul per sq block
    (kills 15/16 of the PE colsum matmuls of the baseline).
  - 1/colsum normalization (DVE; GPSIMD cannot read PSUM) writes attn-out
    into a resident SBUF tile, so the output projection needs no aosc
    round-trip through DRAM; Wout is fully prefetched during attention and
    batch-0's output projection interleaves into batch-1's attention,
    which makes the PE the pacing engine there.
  - Spill DMAs ride the HWDGE (sync) queue: the SWDGE path costs ~1us of
    Pool-engine descriptor generation per DMA and was the long pole of
    every phase-boundary barrier.
"""

import numpy as np

import concourse.bass as bass  # noqa: F401
import concourse.mybir as mybir
import concourse.tile as tile
from concourse import bacc
from concourse.bass_utils import run_bass_kernel_spmd

F32 = mybir.dt.float32
BF16 = mybir.dt.bfloat16

NCORES = 8
EPS = 1e-6


def build_program(S, DIM, H, collective=True):
    HD = 128
    assert DIM == H * HD
    HPC = H // NCORES          # heads per core
    CW = HPC * HD              # per-core channel width for q/k/v
    CT = HPC                   # 128-col tiles per projection group
    NT = 2 * S                 # tokens across both batches
    DC = DIM // 128            # contraction chunks
    TBS = 256                  # token block in phase 1
    NTB = NT // TBS
    SQB = min(512, S)          # sq block in attention
    NSQ = S // SQB
    NST = S // 128             # st (key) chunks per batch
    NC2 = NST // 2             # st chunk pairs
    CSUB = 8 if DC % 8 == 0 else DC
    NCS = DC // CSUB

    nc = bacc.Bacc("TRN2", target_bir_lowering=False, debug=False,
                   num_devices=NCORES)

    hT = nc.dram_tensor("hT", [DIM, NT], BF16, kind="ExternalInput")
    wq = nc.dram_tensor("wq", [DIM, CW], BF16, kind="ExternalInput")
    wk = nc.dram_tensor("wk", [DIM, CW], BF16, kind="ExternalInput")
    wv = nc.dram_tensor("wv", [DIM, CW], BF16, kind="ExternalInput")
    bq = nc.dram_tensor("bq", [128, CT], F32, kind="ExternalInput")
    bk = nc.dram_tensor("bk", [128, CT], F32, kind="ExternalInput")
    wqn = nc.dram_tensor("wqn", [128, CT], F32, kind="ExternalInput")
    wkn = nc.dram_tensor("wkn", [128, CT], F32, kind="ExternalInput")
    cossin = nc.dram_tensor("cossin", [2, 128, S], BF16,
                            kind="ExternalInput")
    wout = nc.dram_tensor("wout", [CW, DIM], BF16, kind="ExternalInput")
    outp = nc.dram_tensor("outp", [NT, DIM], BF16,
                           kind="ExternalOutput")

    hT3 = hT.rearrange("(c p) t -> p c t", p=128)

    ExpF = mybir.ActivationFunctionType.Exp
    SqrtF = mybir.ActivationFunctionType.Sqrt

    from contextlib import ExitStack

    with tile.TileContext(nc) as tc:
        with (
            tc.tile_pool(name="dram", bufs=1, space="DRAM") as dram,
            tc.tile_pool(name="persist", bufs=1) as persist,
        ):
            qsc = dram.tile([CW, NT], BF16, tag="qsc")
            ksc = dram.tile([CW, NT], BF16, tag="ksc")
            vsc = dram.tile([NT, CW], BF16, tag="vsc")
            cc_in = dram.tile([2, NT], F32, tag="cc_in")
            cc_out = dram.tile([2, NT], F32, tag="cc_out")
            # broadcast rinv factors [128, S] per (gi, b), staged via DRAM
            rbsc = dram.tile([4, 128, S], BF16, tag="rbsc")

            ones16 = persist.tile([128, 1], BF16, tag="ones16")
            nc.vector.memset(ones16[:], 1.0)

            bq_t = persist.tile([128, CT], F32, tag="bq")
            bk_t = persist.tile([128, CT], F32, tag="bk")
            wqn_t = persist.tile([128, CT], F32, tag="wqn")
            wkn_t = persist.tile([128, CT], F32, tag="wkn")

            def load_hall(hp, tb):
                hall = hp.tile([128, DC, TBS], BF16, tag="hall")
                for cs in range(NCS):
                    nc.sync.dma_start(
                        hall[:, cs * CSUB:(cs + 1) * CSUB, :],
                        hT3[:, cs * CSUB:(cs + 1) * CSUB,
                            tb * TBS:(tb + 1) * TBS])
                return hall

            # ---------------- phase 1a: Q + K projections + ssq ------------
            # hp and the W pool span phases 1a+1b so the pool-close barrier
            # between the passes has nothing slow to wait for, and Wv can
            # stream in during the 1a tail.
            _h_ctx = ExitStack()
            hp = _h_ctx.enter_context(tc.tile_pool(name="hp", bufs=2))
            wpool = _h_ctx.enter_context(tc.tile_pool(name="wpool", bufs=1))
            wq_t = wpool.tile([128, DC, CW], BF16, tag="wq_t")
            wk_t = wpool.tile([128, DC, CW], BF16, tag="wk_t")
            wv_t = wpool.tile([128, DC, CW], BF16, tag="wv_t")
            wq3 = wq.rearrange("(c p) n -> p c n", p=128)
            wk3 = wk.rearrange("(c p) n -> p c n", p=128)
            wv3 = wv.rearrange("(c p) n -> p c n", p=128)

            with (
                tc.tile_pool(name="evp", bufs=2) as evp,
                tc.tile_pool(name="sqp", bufs=8) as sqp_s,
                tc.tile_pool(name="ps1", bufs=6, space="PSUM") as ps1,
                tc.tile_pool(name="pssq", bufs=2, space="PSUM") as pssq,
            ):
                # first small Wq slice, then the h blocks for tb0/tb1, then
                # Wk in two column halves (its first cts are needed earlier
                # than its last)
                # Wq and the first h block stream chunk-aligned so the
                # chunk-major first group can consume them as they arrive
                nc.sync.dma_start(wq_t[:, 0:2, :], wq3[:, 0:2, :])
                hall0 = hp.tile([128, DC, TBS], BF16, tag="hall",
                                name="hall0")
                halls = {0: hall0}
                nc.sync.dma_start(hall0[:, 0:CSUB, :],
                                  hT3[:, 0:CSUB, 0:TBS])
                nc.sync.dma_start(wq_t[:, 2:CSUB, :], wq3[:, 2:CSUB, :])
                for cs in range(1, NCS):
                    nc.sync.dma_start(
                        hall0[:, cs * CSUB:(cs + 1) * CSUB, :],
                        hT3[:, cs * CSUB:(cs + 1) * CSUB, 0:TBS])
                    nc.sync.dma_start(
                        wq_t[:, cs * CSUB:(cs + 1) * CSUB, :],
                        wq3[:, cs * CSUB:(cs + 1) * CSUB, :])
                # biases/norm weights are needed only from the first
                # eviction (~13us in) — keep them off the critical start
                nc.sync.dma_start(bq_t[:], bq[:])
                nc.sync.dma_start(bk_t[:], bk[:])
                nc.sync.dma_start(wqn_t[:], wqn[:])
                nc.sync.dma_start(wkn_t[:], wkn[:])
                halls[1] = load_hall(hp, 1)
                CWH = CW // 2
                nc.sync.dma_start(wk_t[:, :, 0:CWH], wk3[:, :, 0:CWH])
                nc.sync.dma_start(wk_t[:, :, CWH:CW], wk3[:, :, CWH:CW])

                # group order: q0 q1 k0 k1 q2 k2 q3 k3 ... (so the first k
                # group starts only after Wk had time to land)
                groups = [(0, 0), (1, 0), (0, 1), (1, 1)]
                for tb in range(2, NTB):
                    groups += [(tb, 0), (tb, 1)]

                deferred = []
                for gidx, (tb, gi) in enumerate(groups):
                    # stream Wv in mid-pass, one chunk block per group, so
                    # the hall prefetches stay timely
                    if 8 <= gidx < 8 + NCS:
                        cs = gidx - 8
                        nc.sync.dma_start(
                            wv_t[:, cs * CSUB:(cs + 1) * CSUB, :],
                            wv3[:, cs * CSUB:(cs + 1) * CSUB, :])
                    if tb not in halls:
                        halls[tb] = load_hall(hp, tb)
                    hall = halls[tb]
                    wt, bias_t, spill = [(wq_t, bq_t, qsc),
                                         (wk_t, bk_t, ksc)][gi]
                    ssq_ps = pssq.tile([1, TBS], F32, tag="ssq")
                    evq = evp.tile([128, CT, TBS], BF16, tag="evq")
                    if gidx == 0:
                        # chunk-major accumulation for the very first group
                        # (5 concurrent PSUM groups) so the PE consumes Wq
                        # chunks as the startup DMA stream delivers them
                        # instead of stalling until Wq fully lands
                        pqs = [ps1.tile([128, TBS], F32, tag="acc",
                                        name=f"acc{ct}")
                               for ct in range(CT)]
                        for ch in range(DC):
                            for ct in range(CT):
                                nc.tensor.matmul(
                                    pqs[ct][:],
                                    wt[:, ch, ct * 128:(ct + 1) * 128],
                                    hall[:, ch, :],
                                    start=(ch == 0), stop=(ch == DC - 1))
                    else:
                        pqs = None
                    for ct in range(CT):
                        if pqs is not None:
                            pq = pqs[ct]
                        else:
                            pq = ps1.tile([128, TBS], F32, tag="acc")
                            for ch in range(DC):
                                nc.tensor.matmul(
                                    pq[:],
                                    wt[:, ch, ct * 128:(ct + 1) * 128],
                                    hall[:, ch, :],
                                    start=(ch == 0), stop=(ch == DC - 1))
                        if deferred:
                            deferred.pop(0)()
                        nc.vector.tensor_scalar_add(
                            evq[:, ct, :], pq[:], bias_t[:, ct:ct + 1])
                        sqt = sqp_s.tile([128, TBS], BF16, tag="sqt")
                        nc.vector.tensor_mul(
                            sqt[:], evq[:, ct, :], evq[:, ct, :])

                        def emit_ssq(ssq_ps=ssq_ps, sqt=sqt, ct=ct,
                                     evq=evq, tb=tb, spill=spill, gi=gi):
                            nc.tensor.matmul(
                                ssq_ps[:], ones16[:], sqt[:],
                                start=(ct == 0), stop=(ct == CT - 1))
                            if ct == CT - 1:
                                nc.sync.dma_start(
                                    spill.rearrange("(c p) t -> p c t",
                                                    p=128)
                                    [:, :, tb * TBS:(tb + 1) * TBS],
                                    evq[:])
                                stg = evp.tile([1, TBS], F32, tag="stg",
                                               name="stg")
                                nc.vector.tensor_copy(stg[:], ssq_ps[:])
                                nc.gpsimd.dma_start(
                                    cc_in[gi:gi + 1,
                                          tb * TBS:(tb + 1) * TBS],
                                    stg[:])
                        deferred.append(emit_ssq)
                    if gi == 1:
                        halls.pop(tb, None)
                while deferred:
                    deferred.pop(0)()

            # ---------------- phase 1b: V projection -----------------------
            with (
                tc.tile_pool(name="evv", bufs=2) as evvp,
                tc.tile_pool(name="rbw", bufs=1) as rbwp,
                tc.tile_pool(name="psv", bufs=2, space="PSUM") as psv,
            ):
                # allreduce the ssq partials (overlaps the V pass)
                if collective:
                    nc.gpsimd.collective_compute(
                        "AllReduce", mybir.AluOpType.add,
                        replica_groups=[list(range(NCORES))],
                        ins=[cc_in[:].opt()], outs=[cc_out[:].opt()])
                else:
                    nc.gpsimd.dma_start(cc_out[:], cc_in[:])

                def emit_rb():
                    """rinv rows: single-partition chains, spilled to DRAM.
                    rinv = 1/sqrt(ssq*sc1 + sc2), with the HD**-0.5 attn
                    scale folded into the q-side factor."""
                    for gi in range(2):
                        sc1 = (HD / DIM) if gi == 0 else (1.0 / DIM)
                        sc2 = (HD * EPS) if gi == 0 else EPS
                        for b in range(2):
                            row = rbwp.tile([1, S], BF16, tag="rrow",
                                            name="rrow")
                            # gpsimd dma casts f32 -> bf16 on load
                            nc.gpsimd.dma_start(
                                row[:], cc_out[gi:gi + 1, b * S:(b + 1) * S])
                            with nc.allow_low_precision(
                                    reason="rmsnorm factor in bf16"):
                                nc.vector.tensor_scalar(
                                    row[:], row[:], sc1, sc2,
                                    mybir.AluOpType.mult,
                                    mybir.AluOpType.add)
                                nc.scalar.activation(row[:], row[:], SqrtF)
                                rb16 = rbwp.tile([1, S], BF16, tag="rb16",
                                                 name="rb16")
                                nc.vector.reciprocal(rb16[:], row[:])
                            # broadcast now (Pool is idle here) and stage
                            # the full [128, S] tile in DRAM — attention
                            # then just reloads it, no serial broadcast
                            # chain on its critical path
                            rbf = rbwp.tile([128, S], BF16, tag="rbf",
                                            name="rbf")
                            nc.gpsimd.partition_broadcast(rbf[:], rb16[:])
                            nc.sync.dma_start(rbsc[2 * b + gi], rbf[:])
                    # preload the Exp activation table off the critical path
                    dummy = rbwp.tile([1, 2], F32, tag="dummy")
                    nc.vector.memset(dummy[:], 0.0)
                    nc.scalar.activation(dummy[:], dummy[:], ExpF)

                nsub = TBS // 128
                for tb in range(NTB):
                    hall = load_hall(hp, tb)
                    pv = [[psv.tile([128, 320], F32, tag=f"pv{ts}_{i}",
                                    name=f"pv{ts}_{i}")
                           for i in range(CW // 320)]
                          for ts in range(nsub)]
                    for ch in range(DC):
                        for ts in range(nsub):
                            for i in range(CW // 320):
                                nc.tensor.matmul(
                                    pv[ts][i][:],
                                    hall[:, ch, ts * 128:(ts + 1) * 128],
                                    wv_t[:, ch, i * 320:(i + 1) * 320],
                                    start=(ch == 0), stop=(ch == DC - 1))
                    ev = evvp.tile([128, nsub, CW], BF16, tag="ev")
                    for ts in range(nsub):
                        for i in range(CW // 320):
                            # v bias folds into the host-side output bias
                            nc.scalar.copy(
                                ev[:, ts, i * 320:(i + 1) * 320],
                                pv[ts][i][:])
                        vdma = (nc.gpsimd.dma_start if tb >= NTB - 2
                                else nc.sync.dma_start)
                        vdma(
                            vsc.rearrange("(b p) n -> p b n", p=128)
                            [:, tb * nsub + ts, :],
                            ev[:, ts, :])
                    if tb == 0:
                        emit_rb()

            _h_ctx.close()

            # ---------------- phase 3: attention ---------------------------
            with (
                tc.tile_pool(name="aop", bufs=1) as aop,
                tc.tile_pool(name="wop", bufs=1) as wop,
                tc.tile_pool(name="rbb", bufs=1) as rbbp,
                tc.tile_pool(name="prp", bufs=2) as prp,
                tc.tile_pool(name="xrp", bufs=2) as xrp,
                tc.tile_pool(name="vtp", bufs=1) as vtp,
            ):
                # resident attn output [ch-of-head, pair(b*HPC+hh), tok]
                aoall = aop.tile([128, 2 * HPC, S], BF16, tag="aoall")
                woall = wop.tile([128, HPC, DIM], BF16, tag="woall")
                w3o = wout.rearrange("(h p) n -> p h n", p=128)

                # staged rinv broadcasts + rope tables; loads are emitted
                # in 512-col chunks interleaved with pair-0's prep so the
                # first attention matmuls wait ~2us, not ~15us of serial
                # DMA on an empty queue
                # combined [128, 2(gi), S] rinv tiles per batch and one
                # [128, 2, S] cos/sin tile: halves the DMA count at the
                # attention entry, where HWDGE descriptor generation
                # (625ns each, serialized) was the critical path
                rbb2 = {b: rbbp.tile([128, 2, S], BF16, tag=f"rbb{b}",
                                     name=f"rbb{b}")
                        for b in range(2)}
                cst = rbbp.tile([128, 2, S], BF16, tag="cst")
                rb3 = rbsc[:].rearrange("(h g) p s -> p h g s", g=2)
                cs3 = cossin.rearrange("g p s -> p g s")

                def load_tables_head(b):
                    # first 512 columns of each table (they gate the first
                    # prep chunk)
                    s0 = slice(0, 512)
                    nc.sync.dma_start(rbb2[b][:, :, s0], rb3[:, b, :, s0])
                    nc.sync.dma_start(cst[:, :, s0], cs3[:, :, s0])

                def load_tables_rest(b):
                    s1 = slice(512, S)
                    nc.sync.dma_start(rbb2[b][:, :, s1], rb3[:, b, :, s1])
                    nc.sync.dma_start(cst[:, :, s1], cs3[:, :, s1])

                bhs = [(b, hh) for b in range(2) for hh in range(HPC)]

                def prep(i, nchunk=1, post_chunk0=None):
                    """Load + norm + rope q/k and load v for pair i.  With
                    nchunk>1 the chains are column-sliced and interleaved
                    k-first so the first attention matmuls only wait for
                    the first slices."""
                    b, hh = bhs[i]
                    CS2 = S // nchunk
                    chains = []
                    for gi, (spill, wn, xtag) in enumerate(
                            [(qsc, wqn_t, "xq"), (ksc, wkn_t, "xk")]):
                        xt = prp.tile([128, S], BF16, tag="xt", name="xt")
                        tmc = prp.tile([128, S], BF16, tag="tmc",
                                       name="tmc")
                        tms = prp.tile([128, S], BF16, tag="tms",
                                       name="tms")
                        xr = xrp.tile([128, S], BF16, tag=xtag, name=xtag)

                        def emit_chunk(cc, spill=spill, wn=wn, gi=gi,
                                       xt=xt, tmc=tmc, tms=tms, xr=xr,
                                       hh=hh, b=b):
                            sl = slice(cc * CS2, (cc + 1) * CS2)
                            nc.sync.dma_start(
                                xt[:, sl],
                                spill[hh * 128:(hh + 1) * 128,
                                      b * S + cc * CS2:
                                      b * S + (cc + 1) * CS2])
                            nc.vector.tensor_scalar_mul(
                                xt[:, sl], xt[:, sl], wn[:, hh:hh + 1])
                            nc.vector.tensor_mul(xt[:, sl], xt[:, sl],
                                                 rbb2[b][:, gi, sl])
                            nc.vector.tensor_mul(tmc[:, sl], xt[:, sl],
                                                 cst[:, 0, sl])
                            nc.vector.tensor_mul(
                                tms[0:64, sl], xt[64:128, sl],
                                cst[64:128, 1, sl])
                            nc.vector.tensor_mul(
                                tms[64:128, sl], xt[0:64, sl],
                                cst[0:64, 1, sl])
                            nc.vector.tensor_add(xr[:, sl], tmc[:, sl],
                                                 tms[:, sl])
                        chains.append((emit_chunk, xr))
                    # k chunk 0 and q chunk 0 first (the first scores
                    # matmul needs both), then the rest of k (stationary
                    # side is consumed across every sq block), then q
                    chains[1][0](0)
                    chains[0][0](0)
                    if post_chunk0 is not None:
                        post_chunk0()
                    for cc in range(1, nchunk):
                        chains[1][0](cc)
                    for cc in range(1, nchunk):
                        chains[0][0](cc)
                    return chains[0][1], chains[1][1]

                # whole-batch V tile: one big 1280B-descriptor load per
                # batch instead of five 256B-descriptor loads per pair
                vtall = {}

                def load_vtall(b):
                    vtall[b] = vtp.tile([128, NST, CW], BF16, tag="vtall",
                                        name="vtall")
                    hc = NST // 2
                    v3 = vsc[b * S:(b + 1) * S, :] \
                        .rearrange("(c p) n -> p c n", p=128)
                    nc.sync.dma_start(vtall[b][:, 0:hc, :], v3[:, 0:hc, :])
                    nc.sync.dma_start(vtall[b][:, hc:NST, :],
                                      v3[:, hc:NST, :])

                load_tables_head(0)
                preps = {0: prep(0, nchunk=4,
                                 post_chunk0=lambda: load_tables_rest(0))}
                load_vtall(0)
                # second batch's rinv tables
                nc.sync.dma_start(rbb2[1][:], rb3[:, 1, :, :])

                def prep_units(i):
                    """Like prep(), but returns (qr, kr, units): four
                    emission callbacks [k0, k1, q0, q1] (half-chains) to be
                    spread across the previous pair's sq blocks, so the
                    prep DVE work never delays that pair's colsum tree."""
                    b, hh = bhs[i]
                    CS2 = S // 2
                    tiles = {}
                    for gi, xtag in ((0, "xq"), (1, "xk")):
                        xt = prp.tile([128, S], BF16, tag="xt", name="xt")
                        tmc = prp.tile([128, S], BF16, tag="tmc",
                                       name="tmc")
                        tms = prp.tile([128, S], BF16, tag="tms",
                                       name="tms")
                        xr = xrp.tile([128, S], BF16, tag=xtag, name=xtag)
                        tiles[gi] = (xt, tmc, tms, xr)

                    def unit(gi, cc):
                        spill, wn = [(qsc, wqn_t), (ksc, wkn_t)][gi]
                        xt, tmc, tms, xr = tiles[gi]
                        sl = slice(cc * CS2, (cc + 1) * CS2)
                        nc.sync.dma_start(
                            xt[:, sl],
                            spill[hh * 128:(hh + 1) * 128,
                                  b * S + cc * CS2:b * S + (cc + 1) * CS2])
                        nc.vector.tensor_scalar_mul(
                            xt[:, sl], xt[:, sl], wn[:, hh:hh + 1])
                        nc.vector.tensor_mul(xt[:, sl], xt[:, sl],
                                             rbb2[b][:, gi, sl])
                        nc.vector.tensor_mul(tmc[:, sl], xt[:, sl],
                                             cst[:, 0, sl])
                        nc.vector.tensor_mul(tms[0:64, sl], xt[64:128, sl],
                                             cst[64:128, 1, sl])
                        nc.vector.tensor_mul(tms[64:128, sl], xt[0:64, sl],
                                             cst[0:64, 1, sl])
                        nc.vector.tensor_add(xr[:, sl], tmc[:, sl],
                                             tms[:, sl])

                    units = [lambda gi=gi, cc=cc: unit(gi, cc)
                             for gi, cc in
                             ((1, 0), (1, 1), (0, 0), (0, 1))]
                    return tiles[0][3], tiles[1][3], units

                # (pair, sqb) -> prep-unit callbacks to emit there
                sched = {}

                with (
                    tc.tile_pool(name="etp", bufs=5) as etp,
                    tc.tile_pool(name="trp1", bufs=4) as trp1,
                    tc.tile_pool(name="trp2", bufs=1) as trp2,
                    tc.tile_pool(name="rcp", bufs=2) as rcp,
                ):
                    _ps_ctx = ExitStack()
                    ps_sc = _ps_ctx.enter_context(
                        tc.tile_pool(name="ps_sc", bufs=2, space="PSUM"))
                    ps_av = _ps_ctx.enter_context(
                        tc.tile_pool(name="ps_av", bufs=2, space="PSUM"))
                    ps_cs = _ps_ctx.enter_context(
                        tc.tile_pool(name="ps_cs", bufs=2, space="PSUM"))
                    NTT = NT // 128
                    TPB = NTT // 2
                    ONB = DIM // 512
                    oep = None
                    ps4 = None

                    def emit_outproj_tile(tt):
                        b4 = tt // TPB
                        tloc = tt % TPB
                        tsl = slice(tloc * 128, (tloc + 1) * 128)
                        for nb in range(ONB):
                            wsl = woall[:, :, nb * 512:(nb + 1) * 512]
                            po = ps4.tile([128, 512], F32, tag="po")
                            for hh4 in range(HPC):
                                nc.tensor.matmul(
                                    po[:], aoall[:, b4 * HPC + hh4, tsl],
                                    wsl[:, hh4, :],
                                    start=(hh4 == 0),
                                    stop=(hh4 == HPC - 1))
                            ot = oep.tile([128, 512], BF16, tag="ot")
                            if nb % 2 == 0:
                                nc.vector.tensor_copy(ot[:], po[:])
                            else:
                                nc.scalar.copy(ot[:], po[:])
                            nc.sync.dma_start(
                                outp[tt * 128:(tt + 1) * 128,
                                     nb * 512:(nb + 1) * 512], ot[:])

                    def sqblock(i, b, hh, sqb, qr, kr, vt, two_chunk):
                        """One sq block of attention for pair (b, hh)."""
                        qsl = slice(sqb * SQB, (sqb + 1) * SQB)
                        hsl = slice(hh * 128, (hh + 1) * 128)
                        av = ps_av.tile([128, SQB], F32, tag="av")
                        cs = ps_cs.tile([1, SQB], F32, tag="cs")
                        ets = {}
                        lvl1 = []
                        if two_chunk:
                            for c2 in range(NC2 + 1):
                                if c2 < NC2:
                                    sc2 = ps_sc.tile([128, 2, SQB], F32,
                                                     tag="sc2")
                                    nc.tensor.matmul(
                                        sc2[:, 0, :],
                                        kr[:, (2 * c2) * 128:
                                           (2 * c2 + 1) * 128],
                                        qr[:, qsl], start=True, stop=True)
                                    nc.tensor.matmul(
                                        sc2[:, 1, :],
                                        kr[:, (2 * c2 + 1) * 128:
                                           (2 * c2 + 2) * 128],
                                        qr[:, qsl], start=True, stop=True)
                                    et2 = etp.tile([128, 2, SQB], BF16,
                                                   tag="et2")
                                    nc.scalar.activation(et2[:], sc2[:],
                                                         ExpF)
                                    ets[c2] = et2
                                if c2 >= 1:
                                    p = c2 - 1
                                    et2 = ets[p]
                                    nc.tensor.matmul(
                                        av[:], vt[:, 2 * p, hsl],
                                        et2[:, 0, :],
                                        start=(p == 0), stop=False)
                                    nc.tensor.matmul(
                                        av[:], vt[:, 2 * p + 1, hsl],
                                        et2[:, 1, :],
                                        start=False, stop=(p == NC2 - 1))
                                    if p % 2 == 1:
                                        s1 = trp1.tile([128, 2, SQB],
                                                       BF16, tag="s1")
                                        nc.vector.tensor_add(
                                            s1[:], ets.pop(p - 1)[:],
                                            ets.pop(p)[:])
                                        lvl1.append(s1)
                        else:
                            # single-chunk variant: 1-bank score tiles so
                            # the out-projection can share PSUM; exp pairs
                            # pack into the same [128, 2, SQB] tiles as
                            # the two-chunk path (no extra SBUF tags)
                            s1t = []
                            for c1 in range(NST + 1):
                                if c1 < NST:
                                    if c1 % 2 == 0:
                                        ets[c1 // 2] = etp.tile(
                                            [128, 2, SQB], BF16,
                                            tag="et2", name="et2")
                                    sc1 = ps_sc.tile([128, SQB], F32,
                                                     tag="sc2")
                                    nc.tensor.matmul(
                                        sc1[:],
                                        kr[:, c1 * 128:(c1 + 1) * 128],
                                        qr[:, qsl], start=True, stop=True)
                                    nc.scalar.activation(
                                        ets[c1 // 2][:, c1 % 2, :],
                                        sc1[:], ExpF)
                                if c1 >= 1:
                                    p = c1 - 1
                                    nc.tensor.matmul(
                                        av[:], vt[:, p, hsl],
                                        ets[p // 2][:, p % 2, :],
                                        start=(p == 0),
                                        stop=(p == NST - 1))
                                    if p % 2 == 1:
                                        # fold the completed tile's two
                                        # chunks into an s1 slice
                                        m = p // 2
                                        if m % 2 == 0:
                                            s1t.append(trp1.tile(
                                                [128, 2, SQB], BF16,
                                                tag="s1", name="s1"))
                                        et2 = ets.pop(m)
                                        nc.vector.tensor_add(
                                            s1t[m // 2][:, m % 2, :],
                                            et2[:, 0, :], et2[:, 1, :])
                            lvl1 = s1t
                        s2a = trp2.tile([128, 2, SQB], BF16, tag="s2")
                        nc.vector.tensor_add(s2a[:], lvl1[0][:],
                                             lvl1[1][:])
                        s2b = trp2.tile([128, 2, SQB], BF16, tag="s2b")
                        nc.vector.tensor_add(s2b[:], lvl1[2][:],
                                             lvl1[3][:])
                        s3 = trp2.tile([128, 2, SQB], BF16, tag="s3")
                        nc.vector.tensor_add(s3[:], s2a[:], s2b[:])
                        fold = trp2.tile([128, SQB], BF16, tag="fold")
                        nc.vector.tensor_add(fold[:], s3[:, 0, :],
                                             s3[:, 1, :])
                        nc.tensor.matmul(cs[:], ones16[:], fold[:],
                                         start=True, stop=True)
                        rc = rcp.tile([1, SQB], BF16, tag="rc")
                        with nc.allow_low_precision(
                                reason="1/colsum on bf16 attn out"):
                            nc.vector.reciprocal(rc[:], cs[:])
                        rb2 = rcp.tile([128, SQB], BF16, tag="rb2")
                        nc.gpsimd.partition_broadcast(rb2[:], rc[:])
                        # DVE, not gpsimd: GPSIMD cannot read PSUM (av)
                        nc.vector.tensor_mul(
                            aoall[:, b * HPC + hh, qsl], av[:], rb2[:])
                        for cb in sched.pop((i, sqb), []):
                            cb()

                    def emit_preps_for(i):
                        if i + 1 >= len(bhs):
                            return
                        qr1, kr1, units = prep_units(i + 1)
                        preps[i + 1] = (qr1, kr1)
                        # k halves early (every sq block of the next pair
                        # scans all of kr), last q half spills into the
                        # next pair's first block
                        sched[(i, 1)] = [units[0]]
                        sched[(i, 2)] = [units[1]]
                        sched[(i, 3)] = [units[2]]
                        sched[(i + 1, 0)] = \
                            sched.get((i + 1, 0), []) + [units[3]]

                    # ---- batch 0 pairs: plain attention, Act-critical ----
                    for i in range(HPC):
                        b, hh = bhs[i]
                        emit_preps_for(i)
                        # Wout fully prefetched during the b0 pairs (the
                        # interleaved out-projection needs all of it)
                        nc.sync.dma_start(woall[:, i, 0:DIM // 2],
                                          w3o[:, i, 0:DIM // 2])
                        nc.sync.dma_start(woall[:, i, DIM // 2:DIM],
                                          w3o[:, i, DIM // 2:DIM])
                        qr, kr = preps.pop(i)
                        for sqb in range(NSQ):
                            sqblock(i, b, hh, sqb, qr, kr, vtall[0],
                                    two_chunk=True)

                    # ---- batch 1 pairs: 1-bank score tiles; batch 0's
                    # out-projection interleaves on the PE, which becomes
                    # the pacing engine here ----
                    _ps_ctx.close()
                    _ps_ctx = ExitStack()
                    ps_sc = _ps_ctx.enter_context(
                        tc.tile_pool(name="ps_sc1", bufs=2, space="PSUM"))
                    ps_av = _ps_ctx.enter_context(
                        tc.tile_pool(name="ps_av1", bufs=2, space="PSUM"))
                    ps_cs = _ps_ctx.enter_context(
                        tc.tile_pool(name="ps_cs1", bufs=2, space="PSUM"))
                    ps4 = _ps_ctx.enter_context(
                        tc.tile_pool(name="ps4", bufs=2, space="PSUM"))
                    oep = _ps_ctx.enter_context(
                        tc.tile_pool(name="oe", bufs=2))

                    NB1 = HPC * NSQ          # b1 sq blocks
                    emitted = 0
                    bidx = 0
                    for i in range(HPC, 2 * HPC):
                        b, hh = bhs[i]
                        if i == HPC:
                            load_vtall(1)
                        emit_preps_for(i)
                        qr, kr = preps.pop(i)
                        for sqb in range(NSQ):
                            sqblock(i, b, hh, sqb, qr, kr, vtall[1],
                                    two_chunk=False)
                            # one out-proj tile after every block except a
                            # mid-pair slot (sqb==1) in the first 4 pairs:
                            # pair-boundary blocks MUST emit one, or the PE
                            # stalls on the next pair's q prep chain
                            if not (sqb == 1 and i - HPC < 4) \
                                    and emitted < TPB:
                                emit_outproj_tile(emitted)
                                emitted += 1
                            bidx += 1

                    _ps_ctx.close()

                # ---- remaining out-projection (batch 1 tokens), with the
                # attention pools closed so the eviction ring gets SBUF ----
                with (
                    tc.tile_pool(name="oe2", bufs=6) as oep,
                    tc.tile_pool(name="ps4b", bufs=4, space="PSUM") as ps4,
                ):
                    for tt in range(emitted, NTT):
                        emit_outproj_tile(tt)
    nc.finalize()
    return nc


_PROGRAM_CACHE = {}


def _get_program(S, DIM, H):
    key = (S, DIM, H)
    if key not in _PROGRAM_CACHE:
        _PROGRAM_CACHE[key] = build_program(S, DIM, H)
    return _PROGRAM_CACHE[key]


def _bf16(x):
    import ml_dtypes
    return np.ascontiguousarray(np.asarray(x).astype(ml_dtypes.bfloat16))


def make_in_maps(S, DIM, H, hidden_cond, hidden_uncond, cos_freqs, sin_freqs,
                 Wqkv, bqkv, wq_norm, wk_norm, Wout, bout):
    HD = 128
    HPC = H // NCORES
    CW = HPC * HD
    h = np.concatenate([np.asarray(hidden_cond), np.asarray(hidden_uncond)],
                       axis=0).reshape(2 * S, DIM)
    hT = _bf16(h.T)
    cosT = np.asarray(cos_freqs).T
    sinT = np.asarray(sin_freqs).T  # [128, S]
    HF = HD // 2
    sinrT = np.concatenate([sinT[HF:], -sinT[:HF]], axis=0)
    cossin = _bf16(np.stack([cosT, sinrT], axis=0))  # [2, 128, S]
    Wqkv = np.asarray(Wqkv)
    bqkv = np.asarray(bqkv)
    wq_norm = np.asarray(wq_norm)
    wk_norm = np.asarray(wk_norm)
    Wout = np.asarray(Wout)

    in_maps = []
    for c in range(NCORES):
        sl = slice(c * CW, (c + 1) * CW)
        bq_c = bqkv[0 * DIM:1 * DIM][sl].reshape(HPC, HD).T
        bk_c = bqkv[1 * DIM:2 * DIM][sl].reshape(HPC, HD).T
        in_maps.append({
            "hT": hT,
            "wq": _bf16(Wqkv[:, 0 * DIM:1 * DIM][:, sl]),
            "wk": _bf16(Wqkv[:, 1 * DIM:2 * DIM][:, sl]),
            "wv": _bf16(Wqkv[:, 2 * DIM:3 * DIM][:, sl]),
            "bq": np.ascontiguousarray(bq_c, dtype=np.float32),
            "bk": np.ascontiguousarray(bk_c, dtype=np.float32),
            "wqn": np.ascontiguousarray(
                wq_norm[sl].reshape(HPC, HD).T, dtype=np.float32),
            "wkn": np.ascontiguousarray(
                wk_norm[sl].reshape(HPC, HD).T, dtype=np.float32),
            "cossin": cossin,
            "wout": _bf16(Wout[sl, :]),
        })
    return in_maps


def run(S, DIM, H, inputs):
    nc = _get_program(S, DIM, H)
    in_maps = make_in_maps(S, DIM, H, **inputs)
    res = run_bass_kernel_spmd(nc, in_maps, list(range(NCORES)))
    partial = np.zeros((2 * S, DIM), np.float64)
    for r in res.results:
        partial += r["outp"].astype(np.float64)
    # v-bias contribution: softmax rows sum to 1, so attn(v + bv) =
    # attn(v) + bv, and bv flows through Wout as a constant channel term
    bv_full = np.asarray(inputs["bqkv"])[2 * DIM:3 * DIM].astype(np.float64)
    const_bias = bv_full @ np.asarray(inputs["Wout"]).astype(np.float64) \
        + np.asarray(inputs["bout"])
    out = (partial + const_bias[None, :]).astype(np.float32)
    out = out.reshape(2, 1, S, DIM)
    return out[0], out[1]


def kernel(hidden_cond, hidden_uncond, cos_freqs, sin_freqs,
           Wqkv, bqkv, wq_norm, wk_norm, Wout, bout):
    B, S, DIM = np.asarray(hidden_cond).shape
    assert B == 1
    H = DIM // 128
    return run(S, DIM, H, dict(
        hidden_cond=hidden_cond, hidden_uncond=hidden_uncond,
        cos_freqs=cos_freqs, sin_freqs=sin_freqs, Wqkv=Wqkv, bqkv=bqkv,
        wq_norm=wq_norm, wk_norm=wk_norm, Wout=Wout, bout=bout))
